# revision 2
# baseline (speedup 1.0000x reference)
"""Trainium2 Bass kernel for nn_DualWeightAttention (B=2, S=2048, H=2048, 16 heads).

Sharding: tensor-parallel over heads — 2 heads per core on 8 cores.
Each core computes q/k/v projections for its 2 heads, attention for those
heads (both batches), and a partial output projection against its 256-row
slice of Wo.T. The 8 partial [4096, 2048] f32 outputs are summed on the host.

On-chip layouts (per core), fp16 operands (same PE rate as bf16, 8x finer
mantissa; every tensor here is O(1)-scaled so range is ample):
  qT, kT  [128(d), head, B*S]  fp16  (head dim on partitions)
  v       [128(s), tile, 256]  fp16  (seq on partitions)
  scoresT [128(k), 2, q]       psum f32, TWO banks per tile: a kt-pair of
                               QK matmuls lands in one tile so the exp
                               evacuates [128,1024] per ACT op (amortizes
                               the 352-cycle ACT fixed cost)
  attn_u  [128(k), kt, 512]    fp16  = exp(scoresT) * exp(maskT)  (host
                               precomputes exp(mask); DVE multiplies kt-pairs
                               [128,1024] fp16 in its 2x mode)
  uT      [128(d), head, S]    fp16  = ((attn_u @ v)^T) * 1/denom
  out     [128(s), 512]        f32   = uT.T @ WoT-slice (2-head accumulate)

Softmax denominator: NO ones-vector matmuls (they burned ~77us of PE).
Instead a 4-level in-place DVE tree folds the DEAD attn slab (PV already
consumed it) over kt, then GpSimd partition_all_reduce (otherwise-idle
engine) reduces across the 128 k-partitions with the result broadcast to
every partition, DVE approximates 1/x in f32, and the PV psum is scaled
on evacuation.

The phase-2 emission is software-pipelined: period i interleaves QK(i+1)
kt-pair matmuls with PV(i) matmuls, then the denominator block, then the
out-projection rows of a finished q-chunk. Phase 2 is elementwise-bound
(ScalarE exp + DVE multiplies), so outproj evacuation is split ~3:1
Scalar:DVE to balance the two engines.
"""

import numpy as np

import concourse.mybir as mybir
import concourse.tile as tile
from concourse import bacc
from concourse import bass_isa
from concourse.bass_utils import run_bass_kernel_spmd

P = 128
B = 2
S = 2048
H = 2048
NH = 16
HD = 128
NCORES = 8
HPC = NH // NCORES  # heads per core
DC = HPC * HD       # d-columns per core
QC = 512            # q-chunk (matmul moving free dim)
HT = H // P         # contraction tiles for projections
SCALE = 1.0 / float(np.sqrt(HD))

F32 = mybir.dt.float32
BF16 = mybir.dt.float16  # fp16 over bf16: same PE/DVE rates, finer mantissa

PROJ_DT = BF16  # hsT + wq/wk/wv
QK_DT = BF16    # qT/kT operands
OUT_DT = BF16   # uT + woT
MASK_DT = BF16
EXP = mybir.ActivationFunctionType.Exp
ADD = mybir.AluOpType.add
MULT = mybir.AluOpType.mult

# of every 4 outproj evacuations, this many go to ScalarE (rest to DVE)
EVAC_SCALAR_OF4 = 3


def build_attention_nc(s=S):
    bs = B * s
    kt_n = s // P   # k tiles per batch
    kp_n = kt_n // 2  # kt pairs
    nq = s // QC    # q chunks per batch
    st_n = s // P   # s tiles per batch (out projection)
    vt_n = bs // P  # v tiles (both batches)

    nc = bacc.Bacc("TRN2", target_bir_lowering=False, debug=False, num_devices=NCORES)
    hsT = nc.dram_tensor("hsT", [H, bs], PROJ_DT, kind="ExternalInput")
    maskT = nc.dram_tensor("maskT", [B, s, s], MASK_DT, kind="ExternalInput")
    wqT = nc.dram_tensor("wqT", [H, DC], PROJ_DT, kind="ExternalInput")
    wkT = nc.dram_tensor("wkT", [H, DC], PROJ_DT, kind="ExternalInput")
    wvT = nc.dram_tensor("wvT", [H, DC], PROJ_DT, kind="ExternalInput")
    woT = nc.dram_tensor("woT", [DC, H], OUT_DT, kind="ExternalInput")
    out = nc.dram_tensor("out", [bs, H], F32, kind="ExternalOutput")

    hsT_r = hsT.ap().rearrange("(o p) t -> p o t", p=P)
    wq_r = wqT.ap().rearrange("(o p) d -> p o d", p=P)
    wk_r = wkT.ap().rearrange("(o p) d -> p o d", p=P)
    wv_r = wvT.ap().rearrange("(o p) d -> p o d", p=P)
    wo_r = woT.ap().rearrange("(h p) j -> p h j", p=P)
    out_r = out.ap().rearrange("(t p) j -> p t j", p=P)

    with tile.TileContext(nc) as tc:
        with (
            tc.tile_pool(name="persist", bufs=1) as persist,
        ):
            qT = persist.tile([P, HPC, bs], QK_DT)
            kT = persist.tile([P, HPC, bs], QK_DT)
            vsb = persist.tile([P, vt_n, DC], BF16)
            wo_sb = persist.tile([P, HPC, H], OUT_DT)

            # evacuation helper: alternate DVE/ACT so neither paces the PE
            def evac(idx, dst, src):
                if idx % 2 == 0:
                    nc.scalar.copy(dst, src)
                else:
                    nc.vector.tensor_copy(dst, src)

            # ---------------- Phase 1: q/k/v projections ----------------
            with (
                tc.tile_pool(name="wpool", bufs=1) as wpool,
                tc.tile_pool(name="hpool", bufs=10) as hpool,
                tc.tile_pool(name="ppsum", bufs=2, space="PSUM") as ppsum,
                tc.tile_pool(name="vpsum", bufs=4, space="PSUM") as vpsum,
            ):
                # DMA order matters at startup: the first q-projection group
                # only needs wq + the first hsT quarter, so issue those first
                # and defer wk/wv/wo behind them.
                wq_sb = wpool.tile([P, HT, DC], PROJ_DT, tag="wq")
                wk_sb = wpool.tile([P, HT, DC], PROJ_DT, tag="wk")
                wv_sb = wpool.tile([P, HT, DC], PROJ_DT, tag="wv")
                # split the wq load so the very first matmul group only
                # waits on a quarter of the weights, not the whole tile
                for _wf in range(4):
                    _wsl = slice(_wf * (HT // 4), (_wf + 1) * (HT // 4))
                    nc.sync.dma_start(wq_sb[:, _wsl], wq_r[:, _wsl])

                NQT = 4
                KOQ = HT // NQT  # hsT streamed as 4 quarter-K tiles per s-chunk
                for sc in range(bs // QC):
                    ssl = slice(sc * QC, (sc + 1) * QC)
                    quarters = []
                    for qf in range(NQT):
                        hst = hpool.tile([P, KOQ, QC], PROJ_DT, tag="hst")
                        nc.sync.dma_start(
                            hst[:], hsT_r[:, qf * KOQ : (qf + 1) * KOQ, ssl]
                        )
                        quarters.append(hst)
                    if sc == 0:
                        nc.sync.dma_start(wk_sb[:], wk_r)
                        nc.sync.dma_start(wv_sb[:], wv_r)
                        nc.sync.dma_start(wo_sb[:], wo_r)

                    def hq(ko):
                        return quarters[ko // KOQ][:, ko % KOQ]

                    ev = sc  # evac engine round-robin
                    for h in range(HPC):
                        for wsb, dstT in ((wq_sb, qT), (wk_sb, kT)):
                            ps = ppsum.tile([P, QC], F32, tag="psqk")
                            for ko in range(HT):
                                nc.tensor.matmul(
                                    ps[:],
                                    wsb[:, ko, h * P : (h + 1) * P],
                                    hq(ko),
                                    start=(ko == 0),
                                    stop=(ko == HT - 1),
                                )
                            evac(ev, dstT[:, h, ssl], ps[:])
                            ev += 1
                    # v: ko-outer over 4 concurrent PSUM groups so each hsT
                    # quarter is consumed once and can be recycled early
                    psvs = []
                    for st in range(QC // P):
                        psv = vpsum.tile([P, DC], F32, tag="psv")
                        psvs.append(psv)
                    for ko in range(HT):
                        for st in range(QC // P):
                            nc.tensor.matmul(
                                psvs[st][:],
                                hq(ko)[:, st * P : (st + 1) * P],
                                wv_sb[:, ko, :],
                                start=(ko == 0),
                                stop=(ko == HT - 1),
                            )
                    for st in range(QC // P):
                        evac(ev, vsb[:, sc * (QC // P) + st, :], psvs[st][:])
                        ev += 1

            # ---------------- Phase 2: attention + output projection ----------------
            with (
                tc.tile_pool(name="mpool", bufs=6) as mpool,
                tc.tile_pool(name="apool", bufs=3) as apool,
                tc.tile_pool(name="upool", bufs=1) as upool,
                tc.tile_pool(name="rpool", bufs=2) as rpool,
                tc.tile_pool(name="opool", bufs=6) as opool,
                tc.tile_pool(name="spsum", bufs=2, space="PSUM") as spsum,
                tc.tile_pool(name="upsum", bufs=2, space="PSUM") as upsum,
                tc.tile_pool(name="opsum", bufs=2, space="PSUM") as opsum,
            ):
                units = [
                    (b, qq, h)
                    for b in range(B)
                    for qq in range(nq)
                    for h in range(HPC)
                ]
                nu = len(units)
                mslabs = {}
                aslabs = {}
                psus = {}
                uTs = {}
                KH = kt_n // 2  # kt tiles per mask half-slab

                def mask_prefetch(i):
                    b, qq, h = units[i]
                    if h == 0 and (b, qq) not in mslabs:
                        halves = []
                        for mh in range(2):
                            ms = mpool.tile([P, KH, QC], MASK_DT, tag="mslab")
                            nc.sync.dma_start(
                                ms[:],
                                maskT.ap()[b].rearrange("(kt p) q -> p kt q", p=P)[
                                    :, mh * KH : (mh + 1) * KH,
                                    qq * QC : (qq + 1) * QC,
                                ],
                            )
                            halves.append(ms)
                        mslabs[(b, qq)] = halves

                def qk_pair(i, j):
                    # two scoresT k-tile matmuls into one 2-bank psum tile;
                    # exp(s+m) = exp(s)*exp(m): ScalarE exp evacuates the
                    # [128,1024] pair in one ACT op, and the mask factor
                    # (host-precomputed exp(mask)) is applied as one fp16
                    # [128,1024] SBUF*SBUF multiply in the DVE's 2x mode
                    b, qq, h = units[i]
                    if j == 0:
                        asl = apool.tile([P, kt_n, QC], BF16, tag="aslab")
                        aslabs[i] = asl
                    asl = aslabs[i]
                    pss = spsum.tile([P, 2, QC], F32, tag="pss")
                    for u in range(2):
                        kt = 2 * j + u
                        nc.tensor.matmul(
                            pss[:, u],
                            kT[:, h, b * s + kt * P : b * s + (kt + 1) * P],
                            qT[:, h, b * s + qq * QC : b * s + (qq + 1) * QC],
                            start=True,
                            stop=True,
                        )
                    nc.scalar.activation(asl[:, 2 * j : 2 * j + 2], pss[:], EXP)
                    ms = mslabs[(b, qq)][(2 * j) // KH]
                    mo = (2 * j) % KH
                    nc.vector.tensor_tensor(
                        asl[:, 2 * j : 2 * j + 2],
                        asl[:, 2 * j : 2 * j + 2],
                        ms[:, mo : mo + 2],
                        MULT,
                    )

                def pv_part(i, kt):
                    b, qq, h = units[i]
                    asl = aslabs[i]
                    if kt == 0:
                        psu = upsum.tile([P, QC], F32, tag="psu")
                        psus[i] = psu
                    nc.tensor.matmul(
                        psus[i][:],
                        vsb[:, b * kt_n + kt, h * P : (h + 1) * P],
                        asl[:, kt],
                        start=(kt == 0),
                        stop=(kt == kt_n - 1),
                    )

                def finish_unit(i):
                    # Softmax denominator + PV normalization. PV has consumed
                    # asl, so fold it over kt IN PLACE with a 4-level DVE
                    # tree (fp16 2x mode), reduce across the 128 k-partitions
                    # on the idle GpSimd (result lands broadcast on all
                    # partitions), take 1/x on the DVE (~51-ULP approx), and
                    # scale the PV accumulator while evacuating it to uT.
                    b, qq, h = units[i]
                    asl = aslabs.pop(i)
                    if b not in uTs:
                        uT_new = upool.tile([P, HPC, s], OUT_DT, tag="uT", name="uT")
                        uTs[b] = uT_new
                    for width in (8, 4, 2, 1):
                        nc.vector.tensor_tensor(
                            asl[:, 0:width],
                            asl[:, 0:width],
                            asl[:, width : 2 * width],
                            ADD,
                        )
                    dsum = rpool.tile([P, QC], F32, tag="dsum")
                    nc.gpsimd.partition_all_reduce(
                        dsum[:], asl[:, 0], channels=P, reduce_op=bass_isa.ReduceOp.add
                    )
                    rbc = rpool.tile([P, QC], F32, tag="rbc")
                    nc.vector.reciprocal_approx_fast(out=rbc[:], in_=dsum[:])
                    nc.vector.tensor_tensor(
                        uTs[b][:, h, qq * QC : (qq + 1) * QC],
                        psus.pop(i)[:],
                        rbc[:],
                        MULT,
                    )

                def outproj_chunk(b, qq):
                    # out-projection rows for q-chunk qq only need uT columns
                    # of that chunk, so emit right after its two heads finish
                    # and let it overlap the next chunk's attention periods
                    uT_b = uTs[b]
                    for st in range(qq * (QC // P), (qq + 1) * (QC // P)):
                        for jc in range(H // QC):
                            pso = opsum.tile([P, QC], F32, tag="pso")
                            for h in range(HPC):
                                nc.tensor.matmul(
                                    pso[:],
                                    uT_b[:, h, st * P : (st + 1) * P],
                                    wo_sb[:, h, jc * QC : (jc + 1) * QC],
                                    start=(h == 0),
                                    stop=(h == HPC - 1),
                                )
                            ot = opool.tile([P, QC], F32, tag="ot")
                            # phase 2 is elementwise-bound: bias the evac
                            # split toward ScalarE (exp) vs DVE (multiplies)
                            # to balance the two engines
                            if (st * (H // QC) + jc) % 4 < EVAC_SCALAR_OF4:
                                nc.scalar.copy(ot[:], pso[:])
                            else:
                                nc.vector.tensor_copy(ot[:], pso[:])
                            nc.sync.dma_start(
                                out_r[:, b * st_n + st, jc * QC : (jc + 1) * QC], ot[:]
                            )
                    if qq == nq - 1:
                        uTs.pop(b)

                # software pipeline: period i interleaves QK(i+1) with PV(i)
                # at kt-pair granularity so the PE fills the exp-paced QK
                # stalls with ready PV work
                mask_prefetch(0)
                for j in range(kp_n):
                    qk_pair(0, j)
                for i in range(nu):
                    if i + 1 < nu:
                        mask_prefetch(i + 1)
                    if i + 2 < nu:
                        mask_prefetch(i + 2)
                    for j in range(kp_n):
                        if i + 1 < nu:
                            qk_pair(i + 1, j)
                        pv_part(i, 2 * j)
                        pv_part(i, 2 * j + 1)
                    finish_unit(i)
                    b, qq, h = units[i]
                    if h == HPC - 1:
                        outproj_chunk(b, qq)

    nc.compile()
    return nc


def make_in_maps(hs, mask, Wq, Wk, Wv, Wo):
    """Host-side prep: transpose/shard the full inputs into per-core maps."""
    bs = hs.shape[0] * hs.shape[1]
    proj_np = np.float16
    out_np = np.float16
    hsT = np.ascontiguousarray(hs.reshape(bs, H).T).astype(proj_np)
    maskT = np.exp(
        np.ascontiguousarray(mask[:, 0].transpose(0, 2, 1))
    ).astype(np.float16)
    in_maps = []
    for c in range(NCORES):
        sl = slice(c * DC, (c + 1) * DC)
        in_maps.append(
            {
                "hsT": hsT,
                "maskT": maskT,
                "wqT": np.ascontiguousarray((Wq[sl] * SCALE).T).astype(proj_np),
                "wkT": np.ascontiguousarray(Wk[sl].T).astype(proj_np),
                "wvT": np.ascontiguousarray(Wv[sl].T).astype(proj_np),
                "woT": np.ascontiguousarray(Wo[:, sl].T).astype(out_np),
            }
        )
    return in_maps


_NC_CACHE = {}


def get_nc(s=S):
    if s not in _NC_CACHE:
        _NC_CACHE[s] = build_attention_nc(s)
    return _NC_CACHE[s]


def run(hs, mask, Wq, Wk, Wv, Wo, trace=False, trace_kwargs=None):
    s = hs.shape[1]
    nc = get_nc(s)
    in_maps = make_in_maps(hs, mask, Wq, Wk, Wv, Wo)
    res = run_bass_kernel_spmd(
        nc,
        in_maps,
        core_ids=list(range(NCORES)),
        trace=trace,
        **(trace_kwargs or {}),
    )
    parts = np.stack([r["out"] for r in res.results])
    full = parts.sum(axis=0, dtype=np.float64).astype(np.float32)
    return full.reshape(hs.shape[0], s, H), res


def kernel(hidden_states, attention_mask, Wq, Wk, Wv, Wo):
    hs = np.asarray(hidden_states, dtype=np.float32)
    mask = np.asarray(attention_mask, dtype=np.float32)
    Wq = np.asarray(Wq, dtype=np.float32)
    Wk = np.asarray(Wk, dtype=np.float32)
    Wv = np.asarray(Wv, dtype=np.float32)
    Wo = np.asarray(Wo, dtype=np.float32)
    out, _ = run(hs, mask, Wq, Wk, Wv, Wo)
    return out
